# revision 3
# baseline (speedup 1.0000x reference)
"""DenseRagged forward: relu(x @ W + b) for x[4M, 64], W[64, 128], b[128].

Data-parallel across 8 NeuronCores (row shards, W/b replicated). v2 design:

  - x is transposed on the HOST to feature-major [64, R] and cast fp32 ->
    float8_e3m4 (4 mantissa bits, range +-15.5: |x| <= ~5.7 fits; quarter
    input HBM traffic). W is prescaled by 256 so its ~0.01-scale entries
    land in e3m4 normal range; the epilogue bias is prescaled to 256*b and
    the host decodes the output by 1/256 (power-of-2 dequant, exact).
  - The core's points are split into halves A/B; SBUF x tiles are
    [128, C] with partitions 0:64 = features of A-points, 64:128 =
    B-points. W is stationary ([64,128] stacked twice); each 512-col
    matmul pair runs CONCURRENTLY in the PE via row tiling
    (tile_position (0,0) / (64,0)), K=64 each -> 2 cols/cycle effective.
  - Feature-major PSUM [128 feats, 512 pts] fp32; epilogue is ONE fused
    op per bank: relu(psum + 256*b) -> fp16 (ScalarE activation / DVE
    tensor_scalar split), written straight to SBUF, then DMA'd out
    feature-major [128, R] fp16 in natural point order (no permutation).
  - No on-device transposes (vs v1: frees ~2048 PE cycles + 2 PSUM copies
    per slab, and drops x traffic 2x, making this purely DMA-bound).

Numerics (simulated on the real seed-0 data): rel absmax err ~4.3e-3
vs fp32 reference (budget 2e-2). Ideal DMA: (32.2 MB in + 129 MB out)
/ 358 GB/s ~= 450 us/core.
"""

import sys

if "/opt/trn_rl_repo" not in sys.path:
    sys.path.insert(0, "/opt/trn_rl_repo")

import numpy as np

N_CORES = 8
IN_F = 64
OUT_F = 128
ROWS_TOTAL = 4_000_000
SLAB = 2048  # SBUF x-tile columns per half (points per slab = 2*SLAB)
N_SLABS_FULL = 123
HALF_COLS = SLAB * N_SLABS_FULL  # 251904 points per half
ROWS_PER_CORE = 2 * HALF_COLS  # 503808

_CACHE = {}


def _build(n_slabs):
    import concourse.mybir as mybir
    import concourse.tile as tile
    from concourse import bacc

    fp32 = mybir.dt.float32
    fp16 = mybir.dt.float16
    f8e3 = mybir.dt.float8e3
    relu = mybir.ActivationFunctionType.Relu
    half = SLAB * n_slabs

    nc = bacc.Bacc("TRN2", target_bir_lowering=False)
    x_d = nc.dram_tensor("xt", [128, half], f8e3, kind="ExternalInput")
    w_d = nc.dram_tensor("wstack", [128, OUT_F], f8e3, kind="ExternalInput")
    b_d = nc.dram_tensor("bcol", [128, 1], fp32, kind="ExternalInput")
    # Feature-major output, 256*relu(xW+b) in fp16: [128 feats, 2*half pts].
    y_d = nc.dram_tensor("y", [128, 2 * half], fp16, kind="ExternalOutput")

    with tile.TileContext(nc) as tc:
        with (
            tc.tile_pool(name="const", bufs=1) as cpool,
            tc.tile_pool(name="xin", bufs=4) as xpool,
            tc.tile_pool(name="yout", bufs=3) as ypool,
            tc.tile_pool(name="ps", bufs=4, space="PSUM") as pspool,
        ):
            w_sb = cpool.tile([128, OUT_F], f8e3)
            nc.sync.dma_start(out=w_sb[:], in_=w_d[:])
            b_sb = cpool.tile([128, 1], fp32)
            nc.sync.dma_start(out=b_sb[:], in_=b_d[:])

            for s in range(n_slabs):
                x_sb = xpool.tile([128, SLAB], f8e3)
                nc.sync.dma_start(out=x_sb[:], in_=x_d[:, SLAB * s : SLAB * (s + 1)])

                ya_sb = ypool.tile([128, SLAB], fp16, tag="ya")
                yb_sb = ypool.tile([128, SLAB], fp16, tag="yb")
                for j in range(SLAB // 512):
                    c0 = 512 * j
                    ps_a = pspool.tile([128, 512], fp32)
                    nc.tensor.matmul(
                        ps_a[:],
                        w_sb[0:64, :],
                        x_sb[0:64, c0 : c0 + 512],
                        start=True,
                        stop=True,
                        tile_position=(0, 0),
                        skip_group_check=True,
                    )
                    ps_b = pspool.tile([128, 512], fp32)
                    nc.tensor.matmul(
                        ps_b[:],
                        w_sb[64:128, :],
                        x_sb[64:128, c0 : c0 + 512],
                        start=True,
                        stop=True,
                        tile_position=(64, 0),
                        skip_group_check=True,
                    )
                    # relu(psum + 256 b) -> fp16 (stores 256*y; host /256)
                    nc.scalar.activation(
                        ya_sb[:, c0 : c0 + 512], ps_a[:], relu, bias=b_sb[:]
                    )
                    nc.vector.tensor_scalar(
                        yb_sb[:, c0 : c0 + 512], ps_b[:], b_sb[:], 0.0,
                        mybir.AluOpType.add, mybir.AluOpType.max,
                    )
                nc.scalar.dma_start(
                    out=y_d[:, SLAB * s : SLAB * (s + 1)], in_=ya_sb[:]
                )
                nc.sync.dma_start(
                    out=y_d[:, half + SLAB * s : half + SLAB * (s + 1)], in_=yb_sb[:]
                )

    nc.finalize()
    return nc


def _get_nc(n_slabs):
    if n_slabs not in _CACHE:
        _CACHE[n_slabs] = _build(n_slabs)
    return _CACHE[n_slabs]


def _run(x, W, b, n_slabs, trace=False, trace_kwargs=None):
    import ml_dtypes
    from concourse.bass_utils import run_bass_kernel_spmd

    e3 = ml_dtypes.float8_e3m4
    nc = _get_nc(n_slabs)
    half = SLAB * n_slabs
    rows_core = 2 * half
    rows_used = min(x.shape[0], N_CORES * rows_core)

    x8 = np.asarray(x, dtype=np.float32).astype(e3)
    pad_rows = N_CORES * rows_core - x8.shape[0]
    if pad_rows > 0:
        x8 = np.concatenate([x8, np.zeros((pad_rows, IN_F), e3)])

    w8 = (np.asarray(W, np.float32) * 256.0).astype(e3)
    wstack = np.ascontiguousarray(np.concatenate([w8, w8], axis=0))
    bcol = np.ascontiguousarray((np.asarray(b, np.float32) * 256.0)[:, None])

    in_maps = []
    for c in range(N_CORES):
        shard = x8[c * rows_core : (c + 1) * rows_core]
        # [128, half]: rows 0:64 = A-half features, 64:128 = B-half.
        xtf = np.empty((128, half), e3)
        xtf[0:64] = shard[:half].T
        xtf[64:128] = shard[half:].T
        in_maps.append(
            {"xt": xtf, "wstack": wstack, "bcol": bcol}
        )

    kw = dict(trace_kwargs or {})
    res = run_bass_kernel_spmd(
        nc, in_maps, core_ids=list(range(N_CORES)), trace=trace, **kw
    )

    out = np.empty((rows_used, OUT_F), np.float32)
    pos = 0
    for c in range(N_CORES):
        arr = res.results[c]["y"]  # [128, rows_core] fp16, 256*y, point-ordered
        take = min(rows_core, rows_used - pos)
        out[pos : pos + take] = arr[:, :take].T.astype(np.float32)
        pos += take
    out *= 1.0 / 256.0
    return out, res


def kernel(x, W, b):
    out, _ = _run(x, W, b, N_SLABS_FULL)
    return out


# revision 10
# speedup vs baseline: 1.5501x; 1.5501x over previous
"""DenseRagged forward: relu(x @ W + b) for x[4M, 64], W[64, 128], b[128].

Data-parallel across 8 NeuronCores (row shards, W/b replicated). v2 design:

  - x is transposed on the HOST to feature-major [64, R] and cast fp32 ->
    float8_e3m4 (4 mantissa bits, range +-15.5: |x| <= ~5.7 fits; quarter
    input HBM traffic). W is prescaled by 256 so its ~0.01-scale entries
    land in e3m4 normal range; the epilogue bias is prescaled to 256*b and
    the host decodes the output by 1/256 (power-of-2 dequant, exact).
  - The core's points are split into halves A/B; SBUF x tiles are
    [128, C] with partitions 0:64 = features of A-points, 64:128 =
    B-points. W is stationary ([64,128] stacked twice); each 512-col
    matmul pair runs CONCURRENTLY in the PE via row tiling
    (tile_position (0,0) / (64,0)), K=64 each -> 2 cols/cycle effective.
  - Feature-major PSUM [128 feats, 512 pts] fp32; epilogue is ONE fused
    op per bank: relu(psum + 256*b) -> fp16 (ScalarE activation / DVE
    tensor_scalar split), written straight to SBUF, then DMA'd out
    feature-major [128, R] fp16 in natural point order (no permutation).
  - No on-device transposes (vs v1: frees ~2048 PE cycles + 2 PSUM copies
    per slab, and drops x traffic 2x, making this purely DMA-bound).

Numerics (simulated on the real seed-0 data): rel absmax err ~4.3e-3
vs fp32 reference (budget 2e-2). Ideal DMA: (32.2 MB in + 129 MB out)
/ 358 GB/s ~= 450 us/core.
"""

import sys

if "/opt/trn_rl_repo" not in sys.path:
    sys.path.insert(0, "/opt/trn_rl_repo")

import numpy as np

N_CORES = 8
IN_F = 64
OUT_F = 128
ROWS_TOTAL = 4_000_000
SLAB = 2048  # SBUF x-tile columns per half (points per slab = 2*SLAB)
N_SLABS_FULL = 123
HALF_COLS = SLAB * N_SLABS_FULL  # 251904 points per half
ROWS_PER_CORE = 2 * HALF_COLS  # 503808

_CACHE = {}


def _build(n_slabs):
    import concourse.mybir as mybir
    import concourse.tile as tile
    from concourse import bacc

    fp32 = mybir.dt.float32
    fp16 = mybir.dt.float16
    f8e3 = mybir.dt.float8e3
    relu = mybir.ActivationFunctionType.Relu
    half = SLAB * n_slabs

    nc = bacc.Bacc("TRN2", target_bir_lowering=False)
    x_d = nc.dram_tensor("xt", [128, half], f8e3, kind="ExternalInput")
    w_d = nc.dram_tensor("wstack", [128, OUT_F], f8e3, kind="ExternalInput")
    b_d = nc.dram_tensor("bcol", [128, 1], fp32, kind="ExternalInput")
    # Feature-major output, 256*relu(xW+b) in fp16: [128 feats, 2 halves,
    # half pts] (dim 1: 0 = A-half points, 1 = B-half points).
    y_d = nc.dram_tensor("y", [128, 2, half], fp16, kind="ExternalOutput")

    with tile.TileContext(nc) as tc:
        with (
            tc.tile_pool(name="const", bufs=1) as cpool,
            tc.tile_pool(name="xin", bufs=6) as xpool,
            tc.tile_pool(name="yout", bufs=3) as ypool,
            tc.tile_pool(name="ps", bufs=4, space="PSUM") as pspool,
        ):
            w_sb = cpool.tile([128, OUT_F], f8e3)
            nc.sync.dma_start(out=w_sb[:], in_=w_d[:])
            b_sb = cpool.tile([128, 1], fp32)
            nc.sync.dma_start(out=b_sb[:], in_=b_d[:])

            for s in range(n_slabs):
                x_sb = xpool.tile([128, SLAB], f8e3)
                nc.sync.dma_start(out=x_sb[:], in_=x_d[:, SLAB * s : SLAB * (s + 1)])

                # [:, 0, :] = A-half (ScalarE), [:, 1, :] = B-half (DVE)
                y_sb = ypool.tile([128, 2, SLAB], fp16)
                for j in range(SLAB // 512):
                    c0 = 512 * j
                    ps_a = pspool.tile([128, 512], fp32)
                    nc.tensor.matmul(
                        ps_a[:],
                        w_sb[0:64, :],
                        x_sb[0:64, c0 : c0 + 512],
                        start=True,
                        stop=True,
                        tile_position=(0, 0),
                        skip_group_check=True,
                    )
                    ps_b = pspool.tile([128, 512], fp32)
                    nc.tensor.matmul(
                        ps_b[:],
                        w_sb[64:128, :],
                        x_sb[64:128, c0 : c0 + 512],
                        start=True,
                        stop=True,
                        tile_position=(64, 0),
                        skip_group_check=True,
                    )
                    # relu(psum + 256 b) -> fp16 (stores 256*y; host /256)
                    nc.scalar.activation(
                        y_sb[:, 0, c0 : c0 + 512], ps_a[:], relu, bias=b_sb[:]
                    )
                    nc.vector.tensor_scalar(
                        y_sb[:, 1, c0 : c0 + 512], ps_b[:], b_sb[:], 0.0,
                        mybir.AluOpType.add, mybir.AluOpType.max,
                    )
                # One out-DMA per slab: [128, 2 halves, SLAB cols] block AP.
                nc.scalar.dma_start(
                    out=y_d[:, :, SLAB * s : SLAB * (s + 1)], in_=y_sb[:]
                )

    nc.finalize()
    return nc


def _get_nc(n_slabs):
    if n_slabs not in _CACHE:
        _CACHE[n_slabs] = _build(n_slabs)
    return _CACHE[n_slabs]


def _run(x, W, b, n_slabs, trace=False, trace_kwargs=None):
    import ml_dtypes
    from concourse.bass_utils import run_bass_kernel_spmd

    e3 = ml_dtypes.float8_e3m4
    nc = _get_nc(n_slabs)
    half = SLAB * n_slabs
    rows_core = 2 * half
    rows_used = min(x.shape[0], N_CORES * rows_core)

    x8 = np.asarray(x, dtype=np.float32).astype(e3)
    pad_rows = N_CORES * rows_core - x8.shape[0]
    if pad_rows > 0:
        x8 = np.concatenate([x8, np.zeros((pad_rows, IN_F), e3)])

    w8 = (np.asarray(W, np.float32) * 256.0).astype(e3)
    wstack = np.ascontiguousarray(np.concatenate([w8, w8], axis=0))
    bcol = np.ascontiguousarray((np.asarray(b, np.float32) * 256.0)[:, None])

    in_maps = []
    for c in range(N_CORES):
        shard = x8[c * rows_core : (c + 1) * rows_core]
        # [128, half]: rows 0:64 = A-half features, 64:128 = B-half.
        xtf = np.empty((128, half), e3)
        xtf[0:64] = shard[:half].T
        xtf[64:128] = shard[half:].T
        in_maps.append(
            {"xt": xtf, "wstack": wstack, "bcol": bcol}
        )

    kw = dict(trace_kwargs or {})
    res = run_bass_kernel_spmd(
        nc, in_maps, core_ids=list(range(N_CORES)), trace=trace, **kw
    )

    out = np.empty((rows_used, OUT_F), np.float32)
    pos = 0
    for c in range(N_CORES):
        # [128, 2, half] fp16, 256*y; A-half then B-half = point order.
        arr = res.results[c]["y"].reshape(128, rows_core)
        take = min(rows_core, rows_used - pos)
        out[pos : pos + take] = arr[:, :take].T.astype(np.float32)
        pos += take
    out *= 1.0 / 256.0
    return out, res


def kernel(x, W, b):
    out, _ = _run(x, W, b, N_SLABS_FULL)
    return out
